# revision 1
# baseline (speedup 1.0000x reference)
"""Trainium2 Bass kernel for nn_ActionRecognitionModel (relu-attention action model).

Math: the model's attention operates on a single-channel feature map Z >= 0
([B,1,T,V]); theta/void/g are outer products of Z's flattening with per-model
weight vectors, so the (VT x VT) relu-attention collapses exactly:

  Z[t,v]   = relu(vw.vel + vb) + relu(jw.joint + jb)          (>= 0)
  zvt      = Z flattened in (v,t) order, length VT = 8576
  s[a]     = sum_f w_theta[f] * zvt[134 f + a]      a in [0,134)
  u[j]     = w_void[j % 64] * s[j // 64]
  scores   = relu(theta @ void) = zvt_i * relu(u_j)           (Z >= 0)
  att[i,f] = w_g[f] * zvt_i * Sp,   Sp = sum_j relu(u_j) zvt_j
  logits   = q * (Sp * sumZ) + r * sumZ + t                   (q,r,t folded params)
  out      = softmax(logits)

Each core computes one batch end-to-end on device (data parallel over B,
replicated 4x across the 8 cores); host only folds parameters and stacks the
two batch rows.
"""

import numpy as np

try:
    import concourse.bass as bass
except ImportError:  # fallback if the axon site hook isn't installed
    import sys

    sys.path.insert(0, "/opt/trn_rl_repo")
    import concourse.bass as bass

import concourse.bacc as bacc
import concourse.tile as tile
from concourse import mybir
from concourse.bass_utils import run_bass_kernel_spmd
from concourse.masks import make_identity

F32 = mybir.dt.float32
AF = mybir.ActivationFunctionType
ALU = mybir.AluOpType
AX = mybir.AxisListType

B, C, T, V, F, NCLS = 2, 4, 128, 67, 64, 100
VT = V * T  # 8576
A = VT // F  # 134

# csts layout ([T, 16] fp32): 0:4 vel chan weights, 4:8 joint chan weights,
# 8 vel bias, 9 joint bias (all broadcast down rows); 10:14 W4 segment-dot
# weights; 14 ones column; 15 w_theta (rows 0:64, rest zero)
N_CSTS = 16

_NC_CACHE = {}


def build_nc():
    nc = bacc.Bacc(None, target_bir_lowering=False)
    vel = nc.dram_tensor("vel", [C, T, V], F32, kind="ExternalInput")
    joint = nc.dram_tensor("joint", [C, T, V], F32, kind="ExternalInput")
    csts = nc.dram_tensor("csts", [T, N_CSTS], F32, kind="ExternalInput")
    qrt = nc.dram_tensor("qrt", [1, 3 * NCLS], F32, kind="ExternalInput")
    probs = nc.dram_tensor("probs", [1, NCLS], F32, kind="ExternalOutput")

    with tile.TileContext(nc) as tc:
        with (
            tc.tile_pool(name="const", bufs=1) as const,
            tc.tile_pool(name="work", bufs=1) as work,
            tc.tile_pool(name="psum", bufs=1, space="PSUM") as psum,
            tc.tile_pool(name="dram", bufs=1, space="DRAM") as dpool,
        ):
            # --- input DMAs spread across the three DMA-capable engines;
            # vel is split so the first Z op can start as early as possible ---
            cs = const.tile([T, N_CSTS], F32, name="cs")
            nc.scalar.dma_start(out=cs[:], in_=csts[:])
            vr = vel[:].rearrange("c t v -> t c v")
            vel_sb = work.tile([T, C, V], F32, name="vel_sb")
            nc.sync.dma_start(out=vel_sb[:, 0:2, :], in_=vr[:, 0:2, :])
            nc.scalar.dma_start(out=vel_sb[:, 2:4, :], in_=vr[:, 2:4, :])
            joint_sb = work.tile([T, C, V], F32, name="joint_sb")
            nc.gpsimd.dma_start(out=joint_sb[:], in_=joint[:].rearrange("c t v -> t c v"))
            qrt_sb = const.tile([1, 3, NCLS], F32, name="qrt_sb")
            nc.sync.dma_start(out=qrt_sb[:], in_=qrt[:].rearrange("o (k n) -> o k n", k=3))

            # --- constants generated on device (no input deps) ---
            ident = const.tile([T, T], F32, name="ident")
            make_identity(nc, ident[:])
            ones67 = const.tile([V, 1], F32, name="ones67")
            nc.vector.memset(ones67[:], 1.0)
            # ACT function-table warmup so LoadActFuncSet is off the critical
            # path; reads cs so Tile orders it after the cs DMA on ACT's queue
            warm = const.tile([1, 1], F32, name="warm")
            nc.scalar.activation(warm[:], cs[0:1, 0:1], AF.Exp)

            # --- Z = relu(vw.vel + vb) + relu(jw.joint + jb), [T, V] t-major ---
            zv = work.tile([T, V], F32, name="zv")
            nc.vector.tensor_scalar_mul(zv[:], vel_sb[:, 0, :], cs[:, 0:1])
            for c in range(1, C):
                nc.vector.scalar_tensor_tensor(
                    zv[:], vel_sb[:, c, :], cs[:, c : c + 1], zv[:],
                    op0=ALU.mult, op1=ALU.add,
                )
            zj = work.tile([T, V], F32, name="zj")
            nc.vector.tensor_scalar_mul(zj[:], joint_sb[:, 0, :], cs[:, 4:5])
            for c in range(1, C):
                nc.vector.scalar_tensor_tensor(
                    zj[:], joint_sb[:, c, :], cs[:, 4 + c : 5 + c], zj[:],
                    op0=ALU.mult, op1=ALU.add,
                )
            zvr = work.tile([T, V], F32, name="zvr")
            nc.vector.tensor_scalar(
                zvr[:], zv[:], cs[:, 8:9], 0.0, op0=ALU.add, op1=ALU.max
            )
            zjr = work.tile([T, V], F32, name="zjr")
            nc.vector.tensor_scalar(
                zjr[:], zj[:], cs[:, 9:10], 0.0, op0=ALU.add, op1=ALU.max
            )

            # --- Z = zvr + zjr is never materialized: both the transpose and
            # the PN matmul accumulate the two halves in PSUM, letting the
            # first transpose overlap the second half of the DVE chain ---
            zt_ps = psum.tile([V, T], F32, name="zt_ps")
            nc.tensor.matmul(zt_ps[:], zvr[:], ident[:], is_transpose=True,
                             start=True, stop=False)
            nc.tensor.matmul(zt_ps[:], zjr[:], ident[:], is_transpose=True,
                             start=False, stop=True)
            Zt = work.tile([V, T], F32, name="Zt")
            nc.vector.tensor_copy(Zt[:], zt_ps[:])

            # --- PN[v, (P0,P1,N0,N1,rowsum)] = Z.T @ [W4 | ones] ---
            pn_ps = psum.tile([V, 5], F32, name="pn_ps")
            nc.tensor.matmul(pn_ps[:], zvr[:], cs[:, 10:15], start=True, stop=False)
            nc.tensor.matmul(pn_ps[:], zjr[:], cs[:, 10:15], start=False, stop=True)
            # stage P and -N in SBUF during the idle round-trip window so the
            # post-s ops are single-PSUM-input and fully fused
            P_sb = work.tile([V, 2], F32, name="P_sb")
            nc.vector.tensor_copy(P_sb[:], pn_ps[:, 0:2])
            negN = work.tile([V, 2], F32, name="negN")
            nc.vector.tensor_scalar_mul(negN[:], pn_ps[:, 2:4], -1.0)
            # sumZ and the (r*sumZ + t) logits term depend only on PN col4 --
            # compute them during the idle round-trip window
            R0 = work.tile([V, 1], F32, name="R0")
            nc.vector.tensor_copy(R0[:], pn_ps[:, 4:5])
            red0_ps = psum.tile([1, 1], F32, name="red0_ps")
            nc.tensor.matmul(red0_ps[:], ones67[:], R0[:], start=True, stop=True)
            SZ_sb = work.tile([1, 1], F32, name="SZ_sb")
            nc.vector.tensor_copy(SZ_sb[:], red0_ps[:])
            lgB = work.tile([1, NCLS], F32, name="lgB")
            nc.vector.scalar_tensor_tensor(
                lgB[:], qrt_sb[:, 1, :], SZ_sb[:], qrt_sb[:, 2, :],
                op0=ALU.mult, op1=ALU.add,
            )
            zdram = dpool.tile([V, T], F32, name="zdram")
            nc.scalar.dma_start(out=zdram[:], in_=Zt[:])
            zview = work.tile([F, A], F32, name="zview")
            nc.scalar.dma_start(
                out=zview[:],
                in_=zdram[:].rearrange("v t -> (v t)").rearrange("(f a) -> f a", a=A),
            )

            # --- s67[v,h] = s[2v+h] = sum_f wth[f] * zview[f, 2v+h] ---
            s_ps = psum.tile([V, 2], F32, name="s_ps")
            zv3 = zview[:].rearrange("f (a2 h) -> f a2 h", h=2)
            for h in range(2):
                nc.tensor.matmul(
                    s_ps[:, h : h + 1], zv3[:, :, h], cs[:F, 15:16],
                    start=True, stop=True,
                )
            # --- Sp partials: sum_h relu(s)*P + relu(-s)*N, fused as
            # max(s,0)*P and min(s,0)*(-N) with accumulator outputs ---
            junk = work.tile([V, 2], F32, name="junk")
            acc1 = work.tile([V, 1], F32, name="acc1")
            nc.vector.scalar_tensor_tensor(
                junk[:], s_ps[:], 0.0, P_sb[:], op0=ALU.max, op1=ALU.mult,
                accum_out=acc1[:],
            )
            junk2 = work.tile([V, 2], F32, name="junk2")
            acc2 = work.tile([V, 1], F32, name="acc2")
            nc.vector.scalar_tensor_tensor(
                junk2[:], s_ps[:], 0.0, negN[:], op0=ALU.min, op1=ALU.mult,
                accum_out=acc2[:],
            )
            R1 = work.tile([V, 1], F32, name="R1")
            nc.vector.tensor_add(R1[:], acc1[:], acc2[:])

            # --- red1 = Sp; combined with the early lgB for the final logits ---
            red1_ps = psum.tile([1, 1], F32, name="red1_ps")
            nc.tensor.matmul(red1_ps[:], ones67[:], R1[:], start=True, stop=True)
            X = work.tile([1, 1], F32, name="X")
            nc.vector.tensor_scalar_mul(X[:], red1_ps[:], SZ_sb[:])
            lg = work.tile([1, NCLS], F32, name="lg")
            nc.vector.scalar_tensor_tensor(
                lg[:], qrt_sb[:, 0, :], X[:], lgB[:],
                op0=ALU.mult, op1=ALU.add,
            )

            # --- softmax (no max-subtraction: logits are O(1) for the spec'd
            # randn*0.1 parameter scale, far from fp32 exp overflow) ---
            e = work.tile([1, NCLS], F32, name="e")
            se = work.tile([1, 1], F32, name="se")
            nc.scalar.activation(e[:], lg[:], AF.Exp, accum_out=se[:])
            rse = work.tile([1, 1], F32, name="rse")
            nc.vector.reciprocal(rse[:], se[:])
            pr = work.tile([1, NCLS], F32, name="pr")
            nc.vector.tensor_scalar_mul(pr[:], e[:], rse[:])
            nc.sync.dma_start(out=probs[:], in_=pr[:])
    nc.compile()
    return nc


def get_nc():
    if "nc" not in _NC_CACHE:
        _NC_CACHE["nc"] = build_nc()
    return _NC_CACHE["nc"]


def make_in_maps(joint_matrix, vel_matrix, vc1_w, vc1_b, vc2_w, vc2_b,
                 sc1_w, sc1_b, sc2_w, sc2_b, w_theta, w_void, w_g,
                 convh_w, convh_b, lin_w, lin_b, n_cores=8):
    f32 = np.float32
    vw = (vc2_w[0, 0] * vc1_w[0]).astype(f32)
    vb = f32(vc2_w[0, 0] * vc1_b[0] + vc2_b[0])
    jw = (sc2_w[0, 0] * sc1_w[0]).astype(f32)
    jb = f32(sc2_w[0, 0] * sc1_b[0] + sc2_b[0])

    wvp = np.maximum(w_void, 0).astype(f32)
    wvn = np.maximum(-w_void, 0).astype(f32)

    csts = np.zeros((T, N_CSTS), f32)
    csts[:, 0:4] = vw
    csts[:, 4:8] = jw
    csts[:, 8] = vb
    csts[:, 9] = jb
    csts[:F, 10] = wvp
    csts[F:, 11] = wvp
    csts[:F, 12] = wvn
    csts[F:, 13] = wvn
    csts[:, 14] = 1.0
    csts[:F, 15] = w_theta

    cw = convh_w @ w_g
    q = (lin_w @ cw) / VT
    r = lin_w.sum(axis=1) / VT
    t = lin_w @ convh_b + lin_b
    qrt = np.concatenate([q, r, t]).reshape(1, 3 * NCLS).astype(f32)

    in_maps = []
    for k in range(n_cores):
        b = k % B
        in_maps.append({
            "vel": np.ascontiguousarray(vel_matrix[b], f32),
            "joint": np.ascontiguousarray(joint_matrix[b], f32),
            "csts": csts,
            "qrt": qrt,
        })
    return in_maps


def kernel(**inputs):
    nc = get_nc()
    in_maps = make_in_maps(**inputs)
    last_exc = None
    for attempt in range(3):
        try:
            res = run_bass_kernel_spmd(nc, in_maps, core_ids=list(range(8)))
            break
        except Exception as exc:  # transient NRT/device hiccups recover on retry
            last_exc = exc
            if attempt == 2:
                raise
            import time

            time.sleep(10)
    out = np.stack([res.results[0]["probs"][0], res.results[1]["probs"][0]])
    return out.astype(np.float32)



# revision 5
# speedup vs baseline: 1.3831x; 1.3831x over previous
"""Trainium2 Bass kernel for nn_ActionRecognitionModel (relu-attention action model).

Math: the model's attention operates on a single-channel feature map Z >= 0
([B,1,T,V]); theta/void/g are outer products of Z's flattening with per-model
weight vectors, so the (VT x VT) relu-attention collapses exactly:

  Z[t,v]   = relu(vw.vel + vb) + relu(jw.joint + jb)          (>= 0)
  zvt      = Z flattened in (v,t) order, length VT = 8576
  s[a]     = sum_f w_theta[f] * zvt[134 f + a]      a in [0,134)
  u[j]     = w_void[j % 64] * s[j // 64]
  scores   = relu(theta @ void) = zvt_i * relu(u_j)           (Z >= 0)
  att[i,f] = w_g[f] * zvt_i * Sp,   Sp = sum_j relu(u_j) zvt_j
  logits   = q * (Sp * sumZ) + r * sumZ + t                   (q,r,t folded params)
  out      = softmax(logits)

With P[al] = sum_m relu(w_void)[m] * zvt[64 al + m] and N[al] likewise for
relu(-w_void), Sp = sum_al relu(s_al) P_al + relu(-s_al) N_al.

Both s and (P, N) are matmul contractions over *different* blockings of the
flat vector (134-blocks vs 64-blocks), which do not coexist in any single
2-D SBUF layout (67 and 128 are coprime).  Instead of materializing Z once
and reshaping through DRAM (costly: two dependent DMAs), the host supplies
each input twice, pre-permuted (pure gather, no arithmetic), so the device
computes Z elementwise directly in the two matmul-friendly layouts:

  T2[p, w] = zvt[134*(p%64) + 67*(p//64) + w]   p in [0,128), w in [0,67)
     -> s[67h + w] = sum_p wth2[p, h] * T2[p, w]          (one matmul pair)
  Z3[q, w] = zvt[4288*(q//64) + 64*w + (q%64)]
     -> P/N[67h + w] from stationary [128,4] on Z3        (one matmul pair)

Each relu op's free accumulator provides the row sums for sumZ.  The device
ships the 2x67 relu-combine partials + 128x2 row sums; the host folds the
final Linear+softmax over the 100 classes (all class-weights are host-folded
scalars q, r, t as before).

Each core computes one batch end-to-end on device (data parallel over B,
replicated 4x across the 8 cores).
"""

import numpy as np

try:
    import concourse.bass as bass
except ImportError:  # fallback if the axon site hook isn't installed
    import sys

    sys.path.insert(0, "/opt/trn_rl_repo")
    import concourse.bass as bass

import concourse.bacc as bacc
import concourse.tile as tile
from concourse import mybir
from concourse.bass_utils import run_bass_kernel_spmd

F32 = mybir.dt.float32
ALU = mybir.AluOpType

B, C, T, V, F, NCLS = 2, 4, 128, 67, 64, 100
VT = V * T  # 8576
A = VT // F  # 134

# csts layout ([128, 6] fp32): cols 0:2 = w_theta stationary for the s matmul
# (col h has w_theta[f] at row 64h+f); cols 2:4 = relu(w_void) stationary for
# P; cols 4:6 = -relu(-w_void) stationary for -N.
N_CSTS = 6

# host-side gather indices for the two device layouts (pure permutations)
_p = np.arange(128)[:, None]
_w = np.arange(V)[None, :]
_jT2 = 134 * (_p % 64) + 67 * (_p // 64) + _w  # [128, 67]
_jZ3 = 4288 * (_p // 64) + 64 * _w + (_p % 64)  # [128, 67]
_T2_T, _T2_V = _jT2 % T, _jT2 // T
_Z3_T, _Z3_V = _jZ3 % T, _jZ3 // T

_NC_CACHE = {}


def build_nc(vw, vb, jw, jb):
    vw = [float(x) for x in vw]
    jw = [float(x) for x in jw]
    vb, jb = float(vb), float(jb)
    nc = bacc.Bacc(None, target_bir_lowering=False)
    velZ3 = nc.dram_tensor("velZ3", [T, C, V], F32, kind="ExternalInput")
    jntZ3 = nc.dram_tensor("jntZ3", [T, C, V], F32, kind="ExternalInput")
    velT2 = nc.dram_tensor("velT2", [T, C, V], F32, kind="ExternalInput")
    jntT2 = nc.dram_tensor("jntT2", [T, C, V], F32, kind="ExternalInput")
    csts = nc.dram_tensor("csts", [T, N_CSTS], F32, kind="ExternalInput")
    outa = nc.dram_tensor("outa", [V, 2], F32, kind="ExternalOutput")
    outb = nc.dram_tensor("outb", [T, 2], F32, kind="ExternalOutput")

    with tile.TileContext(nc) as tc:
        with (
            tc.tile_pool(name="work", bufs=1) as work,
            tc.tile_pool(name="psum", bufs=1, space="PSUM") as psum,
        ):
            # --- input DMAs: the Z3 pair heads the two HWDGE queues (it gates
            # the first DVE chain); velT2 goes via Pool/SWDGE; jntT2 and csts
            # ride second on the HWDGE queues (needed later) ---
            velZ3_sb = work.tile([T, C, V], F32, name="velZ3_sb")
            nc.sync.dma_start(out=velZ3_sb[:], in_=velZ3[:])
            jntZ3_sb = work.tile([T, C, V], F32, name="jntZ3_sb")
            nc.scalar.dma_start(out=jntZ3_sb[:], in_=jntZ3[:])
            cs = work.tile([T, N_CSTS], F32, name="cs")
            nc.sync.dma_start(out=cs[:], in_=csts[:])
            jntT2_sb = work.tile([T, C, V], F32, name="jntT2_sb")
            nc.scalar.dma_start(out=jntT2_sb[:], in_=jntT2[:])
            velT2_sb = work.tile([T, C, V], F32, name="velT2_sb")
            nc.gpsimd.dma_start(out=velT2_sb[:], in_=velT2[:])

            # --- chain3: Z in the Z3 layout (conv weights baked as imms) ---
            z3v = work.tile([T, V], F32, name="z3v")
            nc.vector.tensor_scalar(z3v[:], velZ3_sb[:, 0, :], vw[0], vb,
                                    op0=ALU.mult, op1=ALU.add)
            for c in range(1, C):
                nc.vector.scalar_tensor_tensor(
                    z3v[:], velZ3_sb[:, c, :], vw[c], z3v[:],
                    op0=ALU.mult, op1=ALU.add)
            Z3v = work.tile([T, V], F32, name="Z3v")
            nc.vector.tensor_scalar_max(Z3v[:], z3v[:], 0.0)
            z3j = work.tile([T, V], F32, name="z3j")
            nc.vector.tensor_scalar(z3j[:], jntZ3_sb[:, 0, :], jw[0], jb,
                                    op0=ALU.mult, op1=ALU.add)
            for c in range(1, C):
                nc.vector.scalar_tensor_tensor(
                    z3j[:], jntZ3_sb[:, c, :], jw[c], z3j[:],
                    op0=ALU.mult, op1=ALU.add)
            Z3j = work.tile([T, V], F32, name="Z3j")
            nc.vector.tensor_scalar_max(Z3j[:], z3j[:], 0.0)

            # --- chain2: Z in the T2 layout; the relu accumulators provide
            # the row sums that the host reduces to sumZ ---
            rs_sb = work.tile([T, 2], F32, name="rs_sb")
            t2v = work.tile([T, V], F32, name="t2v")
            nc.vector.tensor_scalar(t2v[:], velT2_sb[:, 0, :], vw[0], vb,
                                    op0=ALU.mult, op1=ALU.add)
            for c in range(1, C):
                nc.vector.scalar_tensor_tensor(
                    t2v[:], velT2_sb[:, c, :], vw[c], t2v[:],
                    op0=ALU.mult, op1=ALU.add)
            T2v = work.tile([T, V], F32, name="T2v")
            nc.vector.tensor_scalar(T2v[:], t2v[:], 0.0, 0.0, op0=ALU.max,
                                    op1=ALU.add, accum_out=rs_sb[:, 0:1])
            t2j = work.tile([T, V], F32, name="t2j")
            nc.vector.tensor_scalar(t2j[:], jntT2_sb[:, 0, :], jw[0], jb,
                                    op0=ALU.mult, op1=ALU.add)
            for c in range(1, C):
                nc.vector.scalar_tensor_tensor(
                    t2j[:], jntT2_sb[:, c, :], jw[c], t2j[:],
                    op0=ALU.mult, op1=ALU.add)
            T2j = work.tile([T, V], F32, name="T2j")
            nc.vector.tensor_scalar(T2j[:], t2j[:], 0.0, 0.0, op0=ALU.max,
                                    op1=ALU.add, accum_out=rs_sb[:, 1:2])

            # --- PE: PN and s contractions, each accumulating the vel/joint
            # halves in PSUM (Z = Zv + Zj never materialized) ---
            pn_ps = psum.tile([V, 4], F32, name="pn_ps")
            nc.tensor.matmul(pn_ps[:], Z3v[:], cs[:, 2:6], start=True, stop=False)
            nc.tensor.matmul(pn_ps[:], Z3j[:], cs[:, 2:6], start=False, stop=True)
            s_ps = psum.tile([V, 2], F32, name="s_ps")
            nc.tensor.matmul(s_ps[:], T2v[:], cs[:, 0:2], start=True, stop=False)
            nc.tensor.matmul(s_ps[:], T2j[:], cs[:, 0:2], start=False, stop=True)

            # --- combine: Sp partials = max(s,0)*P + min(s,0)*(-N), summed
            # over the free dim into outa's two columns ---
            PN_sb = work.tile([V, 4], F32, name="PN_sb")
            nc.vector.tensor_copy(PN_sb[:], pn_ps[:])
            accs = work.tile([V, 2], F32, name="accs")
            junk = work.tile([V, 2], F32, name="junk")
            nc.vector.scalar_tensor_tensor(
                junk[:], s_ps[:], 0.0, PN_sb[:, 0:2], op0=ALU.max, op1=ALU.mult,
                accum_out=accs[:, 0:1])
            junk2 = work.tile([V, 2], F32, name="junk2")
            nc.vector.scalar_tensor_tensor(
                junk2[:], s_ps[:], 0.0, PN_sb[:, 2:4], op0=ALU.min, op1=ALU.mult,
                accum_out=accs[:, 1:2])

            # --- outputs: row sums leave early on the scalar queue, the
            # combine partials on the sync queue ---
            nc.scalar.dma_start(out=outb[:], in_=rs_sb[:])
            nc.sync.dma_start(out=outa[:], in_=accs[:])
    nc.compile()
    return nc


def get_nc(vw, vb, jw, jb):
    key = (tuple(np.float32(x) for x in vw), np.float32(vb),
           tuple(np.float32(x) for x in jw), np.float32(jb))
    if key not in _NC_CACHE:
        _NC_CACHE[key] = build_nc(vw, vb, jw, jb)
    return _NC_CACHE[key]


def _fold(vc1_w, vc1_b, vc2_w, vc2_b, sc1_w, sc1_b, sc2_w, sc2_b,
          w_theta, w_void, w_g, convh_w, convh_b, lin_w, lin_b):
    f32 = np.float32
    vw = (vc2_w[0, 0] * vc1_w[0]).astype(f32)
    vb = f32(vc2_w[0, 0] * vc1_b[0] + vc2_b[0])
    jw = (sc2_w[0, 0] * sc1_w[0]).astype(f32)
    jb = f32(sc2_w[0, 0] * sc1_b[0] + sc2_b[0])

    wvp = np.maximum(w_void, 0).astype(f32)
    wvn = np.maximum(-w_void, 0).astype(f32)
    csts = np.zeros((T, N_CSTS), f32)
    csts[:F, 0] = w_theta
    csts[F:, 1] = w_theta
    csts[:F, 2] = wvp
    csts[F:, 3] = wvp
    csts[:F, 4] = -wvn
    csts[F:, 5] = -wvn

    cw = convh_w @ w_g
    q = (lin_w @ cw) / VT
    r = lin_w.sum(axis=1) / VT
    t = lin_w @ convh_b + lin_b
    return vw, vb, jw, jb, csts, q, r, t


def kernel(**inputs):
    f32 = np.float32
    joint_matrix = inputs.pop("joint_matrix")
    vel_matrix = inputs.pop("vel_matrix")
    vw, vb, jw, jb, csts, q, r, t = _fold(**inputs)
    nc = get_nc(vw, vb, jw, jb)

    per_batch = []
    for b in range(B):
        vel, joint = vel_matrix[b], joint_matrix[b]
        per_batch.append({
            "velZ3": np.ascontiguousarray(vel[:, _Z3_T, _Z3_V].transpose(1, 0, 2), f32),
            "jntZ3": np.ascontiguousarray(joint[:, _Z3_T, _Z3_V].transpose(1, 0, 2), f32),
            "velT2": np.ascontiguousarray(vel[:, _T2_T, _T2_V].transpose(1, 0, 2), f32),
            "jntT2": np.ascontiguousarray(joint[:, _T2_T, _T2_V].transpose(1, 0, 2), f32),
            "csts": csts,
        })
    in_maps = [per_batch[k % B] for k in range(8)]

    last_exc = None
    for attempt in range(3):
        try:
            res = run_bass_kernel_spmd(nc, in_maps, core_ids=list(range(8)))
            break
        except Exception as exc:  # transient NRT/device hiccups recover on retry
            last_exc = exc
            if attempt == 2:
                raise
            import time

            time.sleep(10)

    out = np.zeros((B, NCLS), f32)
    for b in range(B):
        outa = res.results[b]["outa"]  # [67, 2] combine partials
        outb = res.results[b]["outb"]  # [128, 2] row sums
        Sp = f32(outa.sum())
        sumZ = f32(outb.sum())
        logits = q * (Sp * sumZ) + r * sumZ + t
        e = np.exp(logits - logits.max())
        out[b] = e / e.sum()
    return out.astype(f32)


# revision 36
# speedup vs baseline: 1.8870x; 1.3643x over previous
"""Trainium2 Bass kernel for nn_ActionRecognitionModel (relu-attention action model).

Math: the model's attention operates on a single-channel feature map Z >= 0
([B,1,T,V]); theta/void/g are outer products of Z's flattening with per-model
weight vectors, so the (VT x VT) relu-attention collapses exactly:

  Z[t,v]   = relu(vw.vel + vb) + relu(jw.joint + jb)          (>= 0)
  zvt      = Z flattened in (v,t) order, length VT = 8576
  s[a]     = sum_f w_theta[f] * zvt[134 f + a]      a in [0,134)
  u[j]     = w_void[j % 64] * s[j // 64]
  scores   = relu(theta @ void) = zvt_i * relu(u_j)           (Z >= 0)
  att[i,f] = w_g[f] * zvt_i * Sp,   Sp = sum_j relu(u_j) zvt_j
  logits   = q * (Sp * sumZ) + r * sumZ + t                   (q,r,t folded params)
  out      = softmax(logits)

With P[al] = sum_m relu(w_void)[m] * zvt[64 al + m] and N[al] likewise for
relu(-w_void), Sp = sum_al relu(s_al) P_al + relu(-s_al) N_al.

Both s and (P, N) are matmul contractions over *different* blockings of the
flat vector (134-blocks vs 64-blocks), which do not coexist in any single
2-D SBUF layout (67 and 128 are coprime).  Instead of materializing Z once
and reshaping through DRAM (two dependent DMA latencies), the host supplies
each input twice, pre-permuted (pure gather, no arithmetic), so the device
computes Z elementwise directly in the two matmul-friendly layouts:

  T2[p, w] = zvt[134*(p%64) + 67*(p//64) + w]   p in [0,128), w in [0,67)
     -> [s0, s1, -s0, -s1] via a wth stationary  (one matmul pair)
  Z3[q, w] = zvt[4288*(q//64) + 64*w + (q%64)]
     -> [P0, P1, N0, N1] via a w_void stationary (one matmul pair)

The madd chains for all four permuted inputs run on the Pool engine (whose
queue-ordered DMAs let compute start right after the DMA issue slices); the
two T2 relus run on DVE to use its free-dim accumulator for the row sums
that the host reduces to sumZ.  One fused DVE op then computes the Sp
partials max(+-s,0)*[P|N] against pn_ps in PSUM, and a single DMA per
output ships [67] partials; the host folds the final Linear+softmax over
the 100 classes (all class weights are host-folded scalars q, r, t).

Each core computes one batch end-to-end on device (data parallel over B,
replicated 4x across the 8 cores).
"""

import numpy as np

try:
    import concourse.bass as bass
except ImportError:  # fallback if the axon site hook isn't installed
    import sys

    sys.path.insert(0, "/opt/trn_rl_repo")
    import concourse.bass as bass

import concourse.bacc as bacc
import concourse.tile as tile
from concourse import mybir
from concourse.bass_utils import run_bass_kernel_spmd

F32 = mybir.dt.float32
BF16 = mybir.dt.bfloat16
ALU = mybir.AluOpType

B, C, T, V, F, NCLS = 2, 4, 128, 67, 64, 100
VT = V * T  # 8576
A = VT // F  # 134

# csts layout ([128, 8] bf16): cols 0:4 = s-matmul stationary producing
# [s0, s1, -s0, -s1] (col h has +/-w_theta[f] at row 64h+f); cols 4:8 = PN
# stationary producing [P0, P1, N0, N1] (relu(w_void) / relu(-w_void)).
N_CSTS = 8

# host-side gather indices for the two device layouts (pure permutations)
_p = np.arange(128)[:, None]
_w = np.arange(V)[None, :]
_jT2 = 134 * (_p % 64) + 67 * (_p // 64) + _w  # [128, 67]
_jZ3 = 4288 * (_p // 64) + 64 * _w + (_p % 64)  # [128, 67]
_T2_T, _T2_V = _jT2 % T, _jT2 // T
_Z3_T, _Z3_V = _jZ3 % T, _jZ3 // T

_NC_CACHE = {}


def build_nc(vw, vb, jw, jb):
    vw = [float(x) for x in vw]
    jw = [float(x) for x in jw]
    vb, jb = float(vb), float(jb)
    nc = bacc.Bacc(None, target_bir_lowering=False)
    velT2 = nc.dram_tensor("velT2", [T, C, V], BF16, kind="ExternalInput")
    jntT2 = nc.dram_tensor("jntT2", [T, C, V], BF16, kind="ExternalInput")
    velZ3 = nc.dram_tensor("velZ3", [T, C, V], BF16, kind="ExternalInput")
    jntZ3 = nc.dram_tensor("jntZ3", [T, C, V], BF16, kind="ExternalInput")
    csts = nc.dram_tensor("csts", [T, N_CSTS], BF16, kind="ExternalInput")
    outa = nc.dram_tensor("outa", [V, 1], F32, kind="ExternalOutput")
    outb = nc.dram_tensor("outb", [T, 2], BF16, kind="ExternalOutput")

    with tile.TileContext(nc) as tc:
        with (
            tc.tile_pool(name="work", bufs=1) as work,
            tc.tile_pool(name="psum", bufs=1, space="PSUM") as psum,
        ):
            # --- input DMAs: the Pool queue's own DMAs precede its compute
            # (queue-ordered); the other two inputs ride the HWDGE queues ---
            velZ3_sb = work.tile([T, C, V], BF16, name="velZ3_sb")
            nc.gpsimd.dma_start(out=velZ3_sb[:], in_=velZ3[:])
            cs = work.tile([T, N_CSTS], BF16, name="cs")
            nc.gpsimd.dma_start(out=cs[:], in_=csts[:])
            velT2_sb = work.tile([T, C, V], BF16, name="velT2_sb")
            nc.sync.dma_start(out=velT2_sb[:], in_=velT2[:])
            jntT2_sb = work.tile([T, C, V], BF16, name="jntT2_sb")
            nc.scalar.dma_start(out=jntT2_sb[:], in_=jntT2[:])
            jntZ3_sb = work.tile([T, C, V], BF16, name="jntZ3_sb")
            nc.scalar.dma_start(out=jntZ3_sb[:], in_=jntZ3[:])

            # --- all four madd chains on Pool (no fused scalar_tensor_tensor
            # opcode there, so each madd is a mul + add pair; conv weights
            # baked as immediates). T2 chains stop before the relu, which
            # runs on DVE to get the free-dim accumulator for sumZ. ---
            def pool_chain(src, w, b, z_name, relu):
                z = work.tile([T, V], BF16, name=z_name)
                nc.gpsimd.tensor_scalar(z[:], src[:, 0, :], w[0], b,
                                        op0=ALU.mult, op1=ALU.add)
                tmp = work.tile([T, V], BF16, name=z_name + "_t")
                for c in range(1, C):
                    nc.gpsimd.tensor_scalar_mul(tmp[:], src[:, c, :], w[c])
                    nc.gpsimd.tensor_add(z[:], z[:], tmp[:])
                if not relu:
                    return z
                r = work.tile([T, V], BF16, name=z_name + "r")
                nc.gpsimd.tensor_scalar_max(r[:], z[:], 0.0)
                return r

            t2v = pool_chain(velT2_sb, vw, vb, "t2v", relu=False)
            t2j = pool_chain(jntT2_sb, jw, jb, "t2j", relu=False)
            Z3v = pool_chain(velZ3_sb, vw, vb, "z3v", relu=True)
            Z3j = pool_chain(jntZ3_sb, jw, jb, "z3j", relu=True)

            rs_sb = work.tile([T, 2], BF16, name="rs_sb")
            T2v = work.tile([T, V], BF16, name="T2v")
            nc.vector.tensor_scalar(T2v[:], t2v[:], 0.0, 0.0, op0=ALU.max,
                                    op1=ALU.add, accum_out=rs_sb[:, 0:1])
            T2j = work.tile([T, V], BF16, name="T2j")
            nc.vector.tensor_scalar(T2j[:], t2j[:], 0.0, 0.0, op0=ALU.max,
                                    op1=ALU.add, accum_out=rs_sb[:, 1:2])

            # --- PE: s and PN contractions, each accumulating the vel/joint
            # halves in PSUM (Z = Zv + Zj never materialized) ---
            s_ps = psum.tile([V, 4], F32, name="s_ps")
            nc.tensor.matmul(s_ps[:], T2v[:], cs[:, 0:4], start=True, stop=False)
            nc.tensor.matmul(s_ps[:], T2j[:], cs[:, 0:4], start=False, stop=True)
            pn_ps = psum.tile([V, 4], F32, name="pn_ps")
            nc.tensor.matmul(pn_ps[:], Z3v[:], cs[:, 4:8], start=True, stop=False)
            nc.tensor.matmul(pn_ps[:], Z3j[:], cs[:, 4:8], start=False, stop=True)

            # --- combine: Sp partials = sum_cols max(+-s,0) * [P|N]. s is
            # staged to SBUF (it is ready early), so the combine's single
            # PSUM operand can be pn_ps (ready late). ---
            s_sb = work.tile([V, 4], F32, name="s_sb")
            nc.vector.tensor_copy(s_sb[:], s_ps[:])
            accs = work.tile([V, 1], F32, name="accs")
            junk = work.tile([V, 4], F32, name="junk")
            nc.vector.scalar_tensor_tensor(
                junk[:], s_sb[:], 0.0, pn_ps[:], op0=ALU.max, op1=ALU.mult,
                accum_out=accs[:])

            # --- outputs: row sums on the scalar queue, combine partials on
            # the sync queue ---
            nc.scalar.dma_start(out=outb[:], in_=rs_sb[:])
            nc.sync.dma_start(out=outa[:], in_=accs[:])
    nc.compile()
    return nc


def get_nc(vw, vb, jw, jb):
    key = (tuple(np.float32(x) for x in vw), np.float32(vb),
           tuple(np.float32(x) for x in jw), np.float32(jb))
    if key not in _NC_CACHE:
        _NC_CACHE[key] = build_nc(vw, vb, jw, jb)
    return _NC_CACHE[key]


def _fold(vc1_w, vc1_b, vc2_w, vc2_b, sc1_w, sc1_b, sc2_w, sc2_b,
          w_theta, w_void, w_g, convh_w, convh_b, lin_w, lin_b):
    f32 = np.float32
    vw = (vc2_w[0, 0] * vc1_w[0]).astype(f32)
    vb = f32(vc2_w[0, 0] * vc1_b[0] + vc2_b[0])
    jw = (sc2_w[0, 0] * sc1_w[0]).astype(f32)
    jb = f32(sc2_w[0, 0] * sc1_b[0] + sc2_b[0])

    wvp = np.maximum(w_void, 0).astype(f32)
    wvn = np.maximum(-w_void, 0).astype(f32)
    csts = np.zeros((T, N_CSTS), f32)
    csts[:F, 0] = w_theta
    csts[F:, 1] = w_theta
    csts[:F, 2] = -w_theta
    csts[F:, 3] = -w_theta
    csts[:F, 4] = wvp
    csts[F:, 5] = wvp
    csts[:F, 6] = wvn
    csts[F:, 7] = wvn

    cw = convh_w @ w_g
    q = (lin_w @ cw) / VT
    r = lin_w.sum(axis=1) / VT
    t = lin_w @ convh_b + lin_b
    return vw, vb, jw, jb, csts, q, r, t


def kernel(**inputs):
    f32 = np.float32
    joint_matrix = inputs.pop("joint_matrix")
    vel_matrix = inputs.pop("vel_matrix")
    vw, vb, jw, jb, csts, q, r, t = _fold(**inputs)
    nc = get_nc(vw, vb, jw, jb)

    import ml_dtypes

    bf16 = ml_dtypes.bfloat16
    per_batch = []
    for b in range(B):
        vel, joint = vel_matrix[b], joint_matrix[b]
        per_batch.append({
            "velZ3": np.ascontiguousarray(vel[:, _Z3_T, _Z3_V].transpose(1, 0, 2), bf16),
            "jntZ3": np.ascontiguousarray(joint[:, _Z3_T, _Z3_V].transpose(1, 0, 2), bf16),
            "velT2": np.ascontiguousarray(vel[:, _T2_T, _T2_V].transpose(1, 0, 2), bf16),
            "jntT2": np.ascontiguousarray(joint[:, _T2_T, _T2_V].transpose(1, 0, 2), bf16),
            "csts": csts.astype(bf16),
        })
    in_maps = [per_batch[k % B] for k in range(8)]

    last_exc = None
    for attempt in range(3):
        try:
            res = run_bass_kernel_spmd(nc, in_maps, core_ids=list(range(8)))
            break
        except Exception as exc:  # transient NRT/device hiccups recover on retry
            last_exc = exc
            if attempt == 2:
                raise
            import time

            time.sleep(10)

    out = np.zeros((B, NCLS), f32)
    for b in range(B):
        outa = res.results[b]["outa"]  # [67, 1] combine partials
        outb = res.results[b]["outb"]  # [128, 2] row sums (bf16)
        Sp = f32(outa.astype(f32).sum())
        sumZ = f32(outb.astype(f32).sum())
        logits = q * (Sp * sumZ) + r * sumZ + t
        e = np.exp(logits - logits.max())
        out[b] = e / e.sum()
    return out.astype(f32)
